# revision 44
# baseline (speedup 1.0000x reference)
"""Trainium2 Bass kernel for nn_Critic (dense MLP critic, 4 layers + LayerNorms).

Strategy (pure data parallel over 8 NeuronCores):
  - batch B=32768 sharded 8x -> 4096 rows/core; weights replicated.
  - all activations kept feature-major ([features on partitions, batch on
    free dim]) so the contraction dim of every matmul is the partition dim.
  - LayerNorm folded into the matmuls: for y = LN(z; g, beta) @ W.T + b,
      y[o,b] = invs[b]*( (W*g)z[o,b] - mu[b]*rowsum(W*g)[o] ) + (W@beta+b)[o]
    realized as an augmented matmul: activations get one extra K-row
    (-mu[b]) paired with a weight row rowsum(W*g)[o]; then
      h = tanh(invs (.) psum + c)  with c = (W@beta+b) applied as the
    per-partition bias operand of the ScalarE tanh.
  - ScalarE runs ONLY Tanh/Copy/Square (all in one activation-table set,
    zero table reloads — the v1 Sqrt<->Tanh alternation cost ~2.7us per
    switch).  1/sqrt(var) is computed on VectorE with a Quake-style rsqrt
    (int bitcast seed + Newton steps): batch-major [128,4] with 2 Newton
    steps for L1 (DVE ops ~60ns there), free-major [1,512] with 1 Newton
    step for L2/L3 (contiguous DVE ops, no cross-engine ping-pong).
  - L1 stats (mean/var over 2080 features) via bn_stats on a second,
    batch-major copy of z, transposed to batch rows via TensorE.
  - L2/L3 stats via (+-1/512)-ones-vector matmuls on PE (sum and sum-of-
    squares of h).
  - emission is software-pipelined: tile i+1's L1 (DMAs, bn_stats chain,
    matmul groups) is interleaved into tile i's L2..L4 so TensorE has
    independent work during the LayerNorm stats tails; input pools are
    triple-buffered so DMAs prefetch a full tile ahead.
  - fp16 data everywhere (weights, activations), f32 PSUM/statistics.
"""

import os
import sys
import numpy as np

for _p in ("/opt/trn_rl_repo",):
    if os.path.isdir(_p) and _p not in sys.path:
        sys.path.append(_p)

from contextlib import ExitStack

import concourse.bass as bass  # noqa: E402
import concourse.tile as tile  # noqa: E402
from concourse import bacc, mybir  # noqa: E402
from concourse.bass_utils import run_bass_kernel_spmd  # noqa: E402
from concourse.tile_rust import add_dep_helper  # noqa: E402

NCORES = 8
B = 32768
BC = B // NCORES  # rows per core
INPUT_DIM = 2048
HALF = INPUT_DIM // 2
N_ACTIONS = 32
D = INPUT_DIM + N_ACTIONS  # 2080
H = 512
NT = 512  # batch columns per tile
EPS = 1e-5
X_NORM = 50.0
V_NORM = 10.0

F16 = mybir.dt.float16
F32 = mybir.dt.float32
I32 = mybir.dt.int32
AF = mybir.ActivationFunctionType
OP = mybir.AluOpType

K1 = 17  # ceil(D/128); last chunk has 32 data rows + 1 aug row (-mu)
K1_LAST = D - 16 * 128  # 32
RSQRT_MAGIC = 0x5F3759DF


def build_nc(bout: float, bc: int = BC):
    """Build + compile the per-core program. bc = rows per core."""
    ntiles = bc // NT
    assert ntiles * NT == bc

    nc = bacc.Bacc("TRN2", target_bir_lowering=False, debug=False,
                   num_devices=NCORES)

    zr_d = nc.dram_tensor("zr", [bc, D], F16, kind="ExternalInput").ap()
    zt_d = nc.dram_tensor("zt", [D, bc], F16, kind="ExternalInput").ap()
    w1_d = nc.dram_tensor("w1a", [D + 1, H], F16, kind="ExternalInput").ap()
    w2_d = nc.dram_tensor("w2a", [H + 1, H], F16, kind="ExternalInput").ap()
    w3_d = nc.dram_tensor("w3a", [H + 1, H], F16, kind="ExternalInput").ap()
    cb_d = nc.dram_tensor("cb", [128, 12], F32, kind="ExternalInput").ap()
    wo_d = nc.dram_tensor("wout", [H, 1], F16, kind="ExternalInput").ap()
    id_d = nc.dram_tensor("ident", [128, 128], F32, kind="ExternalInput").ap()
    q_d = nc.dram_tensor("q", [1, bc], F32, kind="ExternalOutput").ap()

    with tile.TileContext(nc) as tc:
        _emit(tc, ntiles, bout, zr_d, zt_d, w1_d, w2_d, w3_d, cb_d, wo_d,
              id_d, q_d)

    nc.compile()
    return nc


def _emit(tc, ntiles, bout, zr_d, zt_d, w1_d, w2_d, w3_d, cb_d, wo_d, id_d,
          q_d):
    nc = tc.nc
    with ExitStack() as ctx:
        wp = ctx.enter_context(tc.tile_pool(name="wp", bufs=1))
        zt_p = ctx.enter_context(tc.tile_pool(name="ztp", bufs=3))
        zr_p = ctx.enter_context(tc.tile_pool(name="zrp", bufs=2))
        h_p = ctx.enter_context(tc.tile_pool(name="hp", bufs=2))
        u_p = ctx.enter_context(tc.tile_pool(name="up", bufs=4))
        sq_p = ctx.enter_context(tc.tile_pool(name="sqp", bufs=4))
        bc_p = ctx.enter_context(tc.tile_pool(name="bcp", bufs=3))
        st_p = ctx.enter_context(tc.tile_pool(name="stp", bufs=3))
        ps_y = ctx.enter_context(tc.tile_pool(name="psy", bufs=4, space="PSUM"))
        ps_s = ctx.enter_context(tc.tile_pool(name="pss", bufs=1, space="PSUM"))
        ps_t = ctx.enter_context(tc.tile_pool(name="pst", bufs=2, space="PSUM"))

        # ---- persistent constants / weights ----
        # Tiles are allocated here; the DMAs are emitted by load_weights()
        # AFTER tile 0's input DMAs so the first tile's zr/zt transfers
        # aren't queued behind ~3.3MB of weight traffic on the rings.
        w1main = wp.tile([128, 16, H], F16, tag="w1main")
        w1last = wp.tile([K1_LAST + 1, H], F16, tag="w1last")
        w1 = [w1main[:, k, :] for k in range(16)] + [w1last]
        w2main = wp.tile([128, 4, H], F16, tag="w2main")
        w2aug = wp.tile([1, H], F16, tag="w2aug")
        w2 = [w2main[:, k, :] for k in range(4)] + [w2aug]
        w3main = wp.tile([128, 4, H], F16, tag="w3main")
        w3aug = wp.tile([1, H], F16, tag="w3aug")
        w3 = [w3main[:, k, :] for k in range(4)] + [w3aug]
        cbT = wp.tile([128, 12], F32, tag="cbT")
        wo = wp.tile([128, 4, 1], F16, tag="wo")
        ident = wp.tile([128, 128], F32, tag="ident")

        def load_weights():
            for hlf in range(2):
                nc.sync.dma_start(
                    out=w1main[:, hlf * 8:(hlf + 1) * 8, :],
                    in_=w1_d[hlf * 1024:(hlf + 1) * 1024, :]
                        .rearrange("(k p) h -> p k h", k=8))
            nc.sync.dma_start(out=w1last[:, :],
                              in_=w1_d[2048:2048 + K1_LAST + 1, :])
            for wt, wa, wd in ((w2main, w2aug, w2_d), (w3main, w3aug, w3_d)):
                nc.scalar.dma_start(
                    out=wt[:, :, :],
                    in_=wd[0:H, :].rearrange("(k p) h -> p k h", k=4))
                nc.scalar.dma_start(out=wa[:, :], in_=wd[H:H + 1, :])
            nc.scalar.dma_start(out=cbT[:, :], in_=cb_d[:, :])
            nc.scalar.dma_start(
                out=wo[:, :, :],
                in_=wo_d[:, :].rearrange("(k p) o -> p k o", k=4))
            nc.scalar.dma_start(out=ident[:, :], in_=id_d[:, :])

        onesn = wp.tile([128, 1], F16, tag="onesn")
        nc.vector.memset(onesn[:, :], -1.0 / H)
        onesp = wp.tile([128, 1], F16, tag="onesp")
        nc.vector.memset(onesp[:, :], 1.0 / H)
        boutT = wp.tile([1, 1], F32, tag="boutT")
        nc.vector.memset(boutT[:, :], bout)
        qrow = wp.tile([1, ntiles * NT], F32, tag="qrow")

        def evac(py, bctile, htile, cbcol):
            """h = tanh(invs (.) psum + c) : DVE multiply + ACT tanh+bias."""
            u = u_p.tile([128, NT], F16, tag="u")
            nc.vector.tensor_mul(u[:, :], py[:, :], bctile[:, :])
            nc.scalar.activation(htile[:, :], u[:, :], AF.Tanh, bias=cbcol)

        def bcast(row_ap):
            t = bc_p.tile([128, NT], F32, tag="bc")
            inst = nc.gpsimd.partition_broadcast(t[:, :], row_ap)
            return t, inst

        def quake_rsqrt(v_ap, out_ap, tag, rows=128, nr=2):
            """out = 1/sqrt(v) elementwise; v, out: [rows, w] f32 SBUF."""
            w = v_ap.shape[-1]
            it = st_p.tile([rows, w], I32, tag=f"qi_{tag}", name=f"qi_{tag}")
            tt = st_p.tile([rows, w], F32, tag=f"qt_{tag}", name=f"qt_{tag}")
            nc.vector.tensor_scalar(out=it[:, :], in0=v_ap.bitcast(I32),
                                    scalar1=1, scalar2=None,
                                    op0=OP.arith_shift_right)
            nc.vector.tensor_scalar(out=it[:, :], in0=it[:, :],
                                    scalar1=-1, scalar2=RSQRT_MAGIC,
                                    op0=OP.mult, op1=OP.add)
            y = it[:, :].bitcast(F32)
            for r in range(nr):
                nc.vector.tensor_mul(tt[:, :], y, y)
                nc.vector.tensor_mul(tt[:, :], tt[:, :], v_ap)
                nc.vector.tensor_scalar(out=tt[:, :], in0=tt[:, :],
                                        scalar1=-0.5, scalar2=1.5,
                                        op0=OP.mult, op1=OP.add)
                nc.vector.tensor_mul(out_ap, y, tt[:, :])
                y = out_ap

        # ---------- per-tile emission pieces (software-pipelined) ----------

        def front_stats0(it):
            """Allocate tile state + start input DMAs."""
            bs = it * NT
            fr = {"bs": bs}
            fr["zt16"] = zt_p.tile([K1_LAST + 1, NT], F16, tag="zt16", name="zt16")
            fr["invs1"] = st_p.tile([1, NT], F32, tag="invs1", name="invs1")
            fr["zrt"] = zr_p.tile([128, 4, D], F16, tag="zrall", name="zrall")
            fr["v4"] = st_p.tile([128, 4], F32, tag="v4", name="v4")
            fr["nmu4"] = st_p.tile([128, 4], F32, tag="nmu4", name="nmu4")
            fr["iv4"] = st_p.tile([128, 4], F32, tag="iv4", name="iv4")
            ztmain = zt_p.tile([128, 16, NT], F16, tag="ztmain")
            for hlf in range(2):
                nc.scalar.dma_start(
                    out=ztmain[:, hlf * 8:(hlf + 1) * 8, :],
                    in_=zt_d[hlf * 1024:(hlf + 1) * 1024, bs:bs + NT]
                        .rearrange("(k p) n -> p k n", k=8))
            nc.sync.dma_start(out=fr["zt16"][0:K1_LAST, :],
                              in_=zt_d[2048:2048 + K1_LAST, bs:bs + NT])
            fr["zts"] = [ztmain[:, k, :] for k in range(16)] + [fr["zt16"]]
            return fr

        def front_stats_bch(it, fr, bch, gate=None):
            """bn_stats chain for one 128-row batch chunk.

            gate: optional instruction the DVE bursts must wait for — keeps
            next-tile bn_stats from head-blocking this tile's latency-
            critical LN chains in the DVE FIFO.
            """
            bs = fr["bs"]
            nc.sync.dma_start(
                out=fr["zrt"][:, bch, :],
                in_=zr_d[bs + bch * 128:bs + (bch + 1) * 128, :])
            stats = st_p.tile([128, 5, 6], F32, tag=f"st{bch}")
            zrv = fr["zrt"][:, bch, :].rearrange("p (n s) -> p n s", n=5)
            for i in range(5):
                bi = nc.vector.bn_stats(out=stats[:, i, :], in_=zrv[:, i, :])
                if gate is not None:
                    add_dep_helper(bi.ins, gate.ins,
                                   reason="defer bn burst past LN chain")
            mv = st_p.tile([128, 2], F32, tag=f"mv{bch}")
            nc.vector.bn_aggr(out=mv[:, :], in_=stats[:, :, :])
            nc.vector.tensor_scalar_mul(fr["nmu4"][:, bch:bch + 1], mv[:, 0:1], -1.0)
            nc.vector.tensor_scalar_add(fr["v4"][:, bch:bch + 1], mv[:, 1:2], EPS)

        def front_quake(it, fr):
            """rsqrt + transpose -mu/invs to batch rows + L1 broadcast."""
            quake_rsqrt(fr["v4"][:, :], fr["iv4"][:, :], "l1")
            # per-column PE transposes [128,1] -> [1,128] (engines can't
            # address partition offsets that aren't multiples of 32), all
            # four written into one [1,512] PSUM row via free-dim offsets so
            # a single ACT copy extracts each row.
            pmu = ps_t.tile([1, NT], F32, tag="tpr")
            piv = ps_t.tile([1, NT], F32, tag="tpr")
            for bch in range(4):
                sl = slice(bch * 128, (bch + 1) * 128)
                nc.tensor.transpose(out=pmu[0:1, sl],
                                    in_=fr["nmu4"][:, bch:bch + 1],
                                    identity=ident[:, :])
                nc.tensor.transpose(out=piv[0:1, sl],
                                    in_=fr["iv4"][:, bch:bch + 1],
                                    identity=ident[:, :])
            nc.scalar.activation(fr["zt16"][K1_LAST:K1_LAST + 1, :], pmu[0:1, :],
                                 AF.Copy)
            nc.scalar.activation(fr["invs1"][0:1, :], piv[0:1, :], AF.Copy)

        def front_mm(it, fr, m):
            """One L1 matmul group + evac."""
            if m == 0:
                fr["bc1"], fr["bc1_inst"] = bcast(fr["invs1"][0:1, :])
                fr["h1"] = []
            py = ps_y.tile([128, NT], F32, tag="py")
            msl = slice(m * 128, (m + 1) * 128)
            for k in range(K1):
                nc.tensor.matmul(py[:, :], lhsT=w1[k][:, msl], rhs=fr["zts"][k],
                                 start=(k == 0), stop=(k == K1 - 1))
            ht = h_p.tile([128, NT], F16, tag=f"h1_{m}")
            evac(py, fr["bc1"], ht, cbT[:, m:m + 1])
            fr["h1"].append(ht)

        def back_stats(it, lidx, hcur):
            """LN stats for L2/L3: sums on PE, rsqrt on DVE, back to a row."""
            s1 = ps_s.tile([1, NT], F32, tag="s1")
            s2 = ps_s.tile([1, NT], F32, tag="s2")
            for k in range(4):
                nc.tensor.matmul(s1[:, :], lhsT=onesn[:, :], rhs=hcur[k][:, :],
                                 start=(k == 0), stop=(k == 3))
            for k in range(4):
                sq = sq_p.tile([128, NT], F16, tag="sq")
                nc.vector.tensor_mul(sq[:, :], hcur[k][:, :], hcur[k][:, :])
                nc.tensor.matmul(s2[:, :], lhsT=onesp[:, :], rhs=sq[:, :],
                                 start=(k == 0), stop=(k == 3))
            negmu = h_p.tile([1, NT], F16, tag=f"negmu_{lidx}")
            nc.scalar.activation(negmu[:, :], s1[:, :], AF.Copy)
            # var = s2 - mu^2 in free-major (s1 holds -mu; the square kills
            # the sign; eps is negligible vs var >= ~1e-2), then rsqrt in
            # batch-major [128,4] where DVE ops are ~65ns, via PE transposes.
            vt = st_p.tile([1, NT], F32, tag="vt")
            mu2 = st_p.tile([1, NT], F32, tag="mu2")
            # square the SBUF fp16 copy of -mu (HW allows only ONE PSUM input
            # per DVE instruction, so s1*s1 straight from PSUM is illegal)
            nc.vector.tensor_mul(mu2[:, :], negmu[:, :], negmu[:, :])
            nc.vector.tensor_sub(vt[:, :], s2[:, :], mu2[:, :])
            invs = st_p.tile([1, NT], F32, tag="invs")
            quake_rsqrt(vt[0:1, :], invs[0:1, :], "l23", rows=1, nr=1)
            bct, bct_inst = bcast(invs[0:1, :])
            return negmu, bct, bct_inst

        def back_main(it, lidx, hcur, negmu, bct, wts):
            hnew = []
            for m in range(4):
                py = ps_y.tile([128, NT], F32, tag="py")
                msl = slice(m * 128, (m + 1) * 128)
                for k in range(4):
                    nc.tensor.matmul(py[:, :], lhsT=wts[k][:, msl],
                                     rhs=hcur[k][:, :],
                                     start=(k == 0), stop=False)
                nc.tensor.matmul(py[:, :], lhsT=wts[4][:, msl], rhs=negmu[:, :],
                                 start=False, stop=True)
                ht = h_p.tile([128, NT], F16, tag=f"h_{lidx}_{m}")
                evac(py, bct, ht, cbT[:, lidx * 4 + m:lidx * 4 + m + 1])
                hnew.append(ht)
            return hnew

        def back_l4(it, hcur):
            bs = it * NT
            pq = ps_t.tile([1, NT], F32, tag="tpr")
            for k in range(4):
                nc.tensor.matmul(pq[:, :], lhsT=wo[:, k, :], rhs=hcur[k][:, :],
                                 start=(k == 0), stop=(k == 3))
            nc.scalar.activation(qrow[0:1, bs:bs + NT], pq[:, :], AF.Tanh,
                                 bias=boutT[:, :])

        def emit_front_all(it):
            fr = front_stats0(it)
            for bch in range(4):
                front_stats_bch(it, fr, bch)
            front_quake(it, fr)
            for m in range(4):
                front_mm(it, fr, m)
            return fr

        # ---------- pipelined emission: tile i's L2..L4 interleaved with ----
        # ---------- tile i+1's L1, so PE never drains on the stats tails ----
        pipelined = os.environ.get("KERNEL_PIPELINE", "1") == "1"
        if pipelined:
            fr = front_stats0(0)
            for bch in range(4):
                front_stats_bch(0, fr, bch)
            load_weights()
            front_quake(0, fr)
            for m in range(4):
                front_mm(0, fr, m)
            for it in range(ntiles):
                nxt = None
                if it + 1 < ntiles:
                    nxt = front_stats0(it + 1)
                    front_stats_bch(it + 1, nxt, 0)
                    front_stats_bch(it + 1, nxt, 1)
                st2 = back_stats(it, 1, fr["h1"])
                if nxt is not None:
                    front_stats_bch(it + 1, nxt, 2)
                h2 = back_main(it, 1, fr["h1"], st2[0], st2[1], w2)
                if nxt is not None:
                    front_stats_bch(it + 1, nxt, 3)
                st3 = back_stats(it, 2, h2)
                h3 = back_main(it, 2, h2, st3[0], st3[1], w3)
                if nxt is not None:
                    front_quake(it + 1, nxt)
                    front_mm(it + 1, nxt, 0)
                    front_mm(it + 1, nxt, 1)
                back_l4(it, h3)
                if nxt is not None:
                    front_mm(it + 1, nxt, 2)
                    front_mm(it + 1, nxt, 3)
                    fr = nxt
        else:
            load_weights()
            for it in range(ntiles):
                fr = emit_front_all(it)
                st2 = back_stats(it, 1, fr["h1"])
                h2 = back_main(it, 1, fr["h1"], st2[0], st2[1], w2)
                st3 = back_stats(it, 2, h2)
                h3 = back_main(it, 2, h2, st3[0], st3[1], w3)
                back_l4(it, h3)

        nc.sync.dma_start(out=q_d[:, :], in_=qrow[:, :])


# ---------------- host side ----------------

def host_prep(x, a, g1, beta1, g2, beta2, g3, beta3,
              w1, b1, w2, b2, w3, b3, w_out, b_out):
    """Shared (replicated) tensors + full z arrays; returns dict pieces."""
    f16 = np.float16
    z = np.empty((x.shape[0], D), dtype=f16)
    np.multiply(x[:, :HALF], np.float32(1.0 / X_NORM), out=z[:, :HALF], casting="unsafe")
    np.multiply(x[:, HALF:], np.float32(1.0 / V_NORM), out=z[:, HALF:INPUT_DIM], casting="unsafe")
    z[:, INPUT_DIM:] = a.astype(f16)

    def fold(w, g, beta, b):
        """[in+1, out] fp16: rows 0..in-1 = (w*g).T, row in = rowsum(w*g).
        Returns (folded_weight, c) with c = w@beta + b (fp32)."""
        wg = (w.astype(np.float64) * g.astype(np.float64)[None, :])
        rs = wg.sum(axis=1)
        c = w.astype(np.float64) @ beta.astype(np.float64) + b.astype(np.float64)
        out = np.empty((w.shape[1] + 1, w.shape[0]), dtype=f16)
        out[:w.shape[1]] = wg.T.astype(f16)
        out[w.shape[1]] = rs.astype(f16)
        return out, c.astype(np.float32)

    w1a, c1 = fold(w1, g1, beta1, b1)
    w2a, c2 = fold(w2, g2, beta2, b2)
    w3a, c3 = fold(w3, g3, beta3, b3)
    cb = np.empty((128, 12), np.float32)
    for li, c in enumerate((c1, c2, c3)):
        for m in range(4):
            cb[:, li * 4 + m] = c[m * 128:(m + 1) * 128]
    wout = w_out.T.astype(f16)  # [H, 1]
    bout = float(b_out[0])
    ident = np.eye(128, dtype=np.float32)
    return z, w1a, w2a, w3a, cb, wout, bout, ident


_NC_CACHE = {}


def make_in_maps(z, w1a, w2a, w3a, cb, wout, ident):
    in_maps = []
    for c in range(NCORES):
        zc = z[c * BC:(c + 1) * BC]
        in_maps.append({
            "zr": np.ascontiguousarray(zc),
            "zt": np.ascontiguousarray(zc.T),
            "w1a": w1a, "w2a": w2a, "w3a": w3a, "cb": cb,
            "wout": wout, "ident": ident,
        })
    return in_maps


def kernel(**inputs):
    inputs = {k: np.asarray(v) for k, v in inputs.items()}
    z, w1a, w2a, w3a, cb, wout, bout, ident = host_prep(**inputs)

    key = (round(bout, 10), BC)
    if key not in _NC_CACHE:
        _NC_CACHE[key] = build_nc(bout, BC)
    nc = _NC_CACHE[key]

    in_maps = make_in_maps(z, w1a, w2a, w3a, cb, wout, ident)
    res = run_bass_kernel_spmd(nc, in_maps, list(range(NCORES)))
    q = np.concatenate([res.results[c]["q"].reshape(BC, 1) for c in range(NCORES)],
                       axis=0).astype(np.float32)
    return q
